# revision 2
# baseline (speedup 1.0000x reference)
"""Bass/Trainium2 SPMD kernel for the BipartiteGNN problem — v2.

Differences vs v1 (see kernel.py docstring for the shared architecture):
  - All feature data is bf16 (table, gather tiles, encoder activations);
    accumulation stays f32 in PSUM.  Weight matmuls use a split-weight
    pair (W = W_hi + W_lo, both bf16, accumulated in PSUM) so weight
    quantization error vanishes; measured rel err ~3.5e-3 vs the 2e-2
    gate.
  - The per-group segment sum runs on the Tensor engine as an
    accumulation of identity-stationary matmuls over the gathered K
    slots (+1 for the self contribution), replacing the DVE strided
    tensor_reduce.  The conv bias is folded into the same accumulation
    as a rank-1 outer product sqrt(deg)[dst] x bias[f].
  - The final layer writes node-major [v, H] f32 directly (no
    transpose); conv groups are processed in descending-cap order so the
    drain tail is the smallest group.
  - Constants are packed into three DRAM tensors (bf16 / f32 / int16)
    loaded with one dma_start each, and encoder self-tiles live in one
    [128, G, 128] SBUF array stored with batched strided DMAs — the
    HWDGE device (625 ns per dma_start, serial) was gating the encoder
    head phase.
"""

import numpy as np

N_NODES = 20000
N_EDGES = 640000
N_PLANTS = 10000
H = 128
N_CORES = 8
V_PER_CORE = N_NODES // N_CORES

KCH = 64  # max gather blocks per SWDGE instruction


# ---------------------------------------------------------------------------
# Host-side planning: permutation, group capacities, gather index arrays.
# ---------------------------------------------------------------------------

class Plan:
    __slots__ = (
        "n", "v", "ncores", "ngroups", "caps", "base", "tk",
        "pi", "dinv", "idxs", "dinv_b", "sdeg", "ca",
    )


def build_plan(src, dst, n, v, ncores):
    p = Plan()
    p.n, p.v, p.ncores = n, v, ncores
    G = (v + 127) // 128
    p.ngroups = G

    indeg = np.bincount(dst, minlength=n)
    deg = (indeg + 1).astype(np.float32)
    dinv = (1.0 / np.sqrt(deg)).astype(np.float32)
    p.dinv = dinv

    # Per-core in-degree sort (descending) of the owned rows.
    pi = np.empty(n, np.int64)
    for c in range(ncores):
        lo = c * v
        order = np.argsort(-indeg[lo:lo + v], kind="stable") + lo
        pi[lo:lo + v] = order
    inv_pi = np.empty(n, np.int64)
    inv_pi[pi] = np.arange(n)
    p.pi = pi

    indeg_perm = indeg[pi]

    caps = np.zeros(G, np.int64)
    for g in range(G):
        mx = 0
        for c in range(ncores):
            j0 = g * 128
            j1 = min(j0 + 128, v)
            blk = indeg_perm[c * v + j0: c * v + j1]
            if blk.size:
                mx = max(mx, int(blk.max()))
        caps[g] = max(mx, 1)
    p.caps = tuple(int(x) for x in caps)
    base = np.concatenate([[0], np.cumsum(caps)])
    p.base = base
    p.tk = int(base[-1]) * 128

    gh = G // 2
    vh = 128 * gh

    def table_pos(r):
        c = r // v
        j = r % v
        return np.where(j < vh, c * vh + j,
                        ncores * vh + c * (v - vh) + (j - vh))

    r_dst = inv_pi[dst]
    r_src = inv_pi[src]
    tpos_all = table_pos(r_src)
    # sort each dst's sources by table position, so part-A sources occupy
    # the lowest k slots (enables the AG-A-only head chunks)
    order_e = np.lexsort((tpos_all, r_dst))
    rs = r_dst[order_e]
    ss = r_src[order_e]
    tp = tpos_all[order_e]
    run_start = np.searchsorted(rs, rs, side="left")
    k_e = np.arange(len(rs)) - run_start
    c_e = rs // v
    j_e = rs % v
    g_e = j_e // 128
    p_e = j_e % 128
    pos_e = (base[g_e] + k_e) * 128 + p_e

    tokens = np.full((ncores, p.tk), n, np.int64)
    tokens[c_e, pos_e] = tp

    # per-group guaranteed part-A slot count: min over occupied (core, dst)
    # of that dst's number of part-A sources (table rows < ncores*vh)
    isA = tp < ncores * vh
    key3 = (c_e * G + g_e) * 128 + p_e
    acnt3 = np.bincount(key3[isA], minlength=ncores * G * 128)
    acnt3 = acnt3.reshape(ncores, G, 128)
    ca = np.zeros(G, np.int64)
    for g in range(G):
        # only real dsts count (last group may be short)
        wg = min(128, v - g * 128)
        ca[g] = acnt3[:, g, :wg].min()
    p.ca = tuple(int(x) for x in ca)

    assert tokens.max() <= n and tokens.min() >= 0 and n < 32768
    t16 = tokens.astype(np.int16).reshape(ncores, p.tk // 16, 16)
    t16 = t16.transpose(0, 2, 1)
    p.idxs = np.ascontiguousarray(np.tile(t16, (1, 8, 1)))  # [nc,128,tk/16]

    # dinv in permuted order, [128, G] per core (ACT per-partition scales);
    # sqrt(deg) rows [1, G*128] per core (bias outer-product stationary).
    pad = np.zeros((ncores, G * 128), np.float32)
    spad = np.zeros((ncores, G * 128), np.float32)
    dv = dinv[pi]
    sq = np.sqrt(deg)[pi]
    for c in range(ncores):
        pad[c, :v] = dv[c * v:(c + 1) * v]
        spad[c, :v] = sq[c * v:(c + 1) * v]
    p.dinv_b = np.ascontiguousarray(
        pad.reshape(ncores, G, 128).transpose(0, 2, 1))  # [nc, 128, G]
    p.sdeg = spad.reshape(ncores, 1, G * 128)            # [nc, 1, G*128]
    return p


# ---------------------------------------------------------------------------
# Device program.
# ---------------------------------------------------------------------------

def build_program(n, v, ncores, caps, ca=None, fake_ag=False):
    import concourse.bass as bass
    import concourse.bacc as bacc
    import concourse.mybir as mybir
    import concourse.tile as tile

    F32 = mybir.dt.float32
    BF16 = mybir.dt.bfloat16
    I16 = mybir.dt.int16
    AF = mybir.ActivationFunctionType

    G = (v + 127) // 128
    base = [0]
    for cp in caps:
        base.append(base[-1] + cp)
    tk16 = base[-1] * 128 // 16
    maxcap = max(caps)

    nc = bacc.Bacc("TRN2", target_bir_lowering=False, debug=False,
                   num_devices=ncores)

    # --- I/O ---
    # Packed bf16 consts: w1hi w1lo w2hi w2lo g0hi g0lo g1hi g1lo ident
    # (9 x 128 cols), then rows gb0 gb1 (2 x 128) + sdeg (G*128) on the
    # first partition of a second region appended column-wise.
    NB = 9 * 128
    NROW = 2 * 128 + G * 128
    cb = nc.dram_tensor("cb", [128, NB], BF16, kind="ExternalInput")
    rowp = nc.dram_tensor("rowp", [1, NROW], BF16, kind="ExternalInput")
    cf = nc.dram_tensor("cf", [128, G + 2], F32, kind="ExternalInput")
    idxs = nc.dram_tensor("idxs", [128, tk16], I16, kind="ExternalInput")
    xT = nc.dram_tensor("xT", [H, v], BF16, kind="ExternalInput")
    hout = nc.dram_tensor("hout", [v, H], F32, kind="ExternalOutput")

    # --- internal DRAM --- (bf16 gather tables; +1 zeros row for padding)
    ghalf = G // 2
    vh = 128 * ghalf
    if not (0 < vh < v):
        vh = v
    hws0_locA = nc.dram_tensor("hws0_locA", [vh, H], BF16)
    hws1_locA = nc.dram_tensor("hws1_locA", [vh, H], BF16)
    hws0_locB = (nc.dram_tensor("hws0_locB", [v - vh, H], BF16)
                 if vh < v else None)
    hws1_locB = (nc.dram_tensor("hws1_locB", [v - vh, H], BF16)
                 if vh < v else None)
    hws0_full = nc.dram_tensor("hws0_full", [n + 1, H], BF16,
                               addr_space="Shared")
    hws1_full = nc.dram_tensor("hws1_full", [n + 1, H], BF16,
                               addr_space="Shared")

    rg = [list(range(ncores))]

    with tile.TileContext(nc) as tc:
        with (
            tc.tile_pool(name="const", bufs=1) as cpool,
            tc.tile_pool(name="enc", bufs=4) as epool,
            tc.tile_pool(name="gth", bufs=4) as gpool,
            tc.tile_pool(name="gthA", bufs=10) as gpoolA,
            tc.tile_pool(name="stgb", bufs=3) as bpool,
            tc.tile_pool(name="selfp", bufs=1) as spool,
            tc.tile_pool(name="penc1", bufs=2, space="PSUM") as penc1,
            tc.tile_pool(name="penc2", bufs=1, space="PSUM") as penc2,
            tc.tile_pool(name="pagg", bufs=2, space="PSUM") as pagg,
            tc.tile_pool(name="pmm", bufs=2, space="PSUM") as pmm,
            tc.tile_pool(name="ptp", bufs=1, space="PSUM") as ptp,
        ):
            # ---- packed constants (3 dma_starts + idx) ----
            cbs = cpool.tile([128, NB], BF16, tag="cb")
            nc.sync.dma_start(cbs[:], cb[:, :])
            cfs = cpool.tile([128, G + 2], F32, tag="cf")
            nc.sync.dma_start(cfs[:], cf[:, :])
            xh = []
            for (h0, h1) in ((0, vh), (vh, v)):
                t = cpool.tile([128, max(vh, v - vh)], BF16, tag=f"x{h0}")
                nc.sync.dma_start(t[:, :h1 - h0], xT[:, h0:h1])
                xh.append(t)
            rps = cpool.tile([1, NROW], BF16, tag="rowp")
            nc.sync.dma_start(rps[:], rowp[:, :])
            idxs_sb = cpool.tile([128, tk16], I16, tag="idx")
            nc.sync.dma_start(idxs_sb[:], idxs[:, :])

            def cslice(i):
                return cbs[:, i * 128:(i + 1) * 128]

            w1hi, w1lo = cslice(0), cslice(1)
            w2hi, w2lo = cslice(2), cslice(3)
            g0hi, g0lo = cslice(4), cslice(5)
            g1hi, g1lo = cslice(6), cslice(7)
            ids = cslice(8)
            gb0s = rps[0:1, 0:128]
            gb1s = rps[0:1, 128:256]
            sds = rps[0:1, 256:256 + G * 128]
            dBs = cfs[:, 0:G]
            b1s = cfs[:, G:G + 1]
            b2s = cfs[:, G + 1:G + 2]

            zs = cpool.tile([1, H], BF16, tag="zs")
            nc.vector.memset(zs[:], 0.0)
            nc.sync.dma_start(hws0_full[n:n + 1, :], zs[:])
            nc.sync.dma_start(hws1_full[n:n + 1, :], zs[:])

            # self-contribution arrays, one per layer: [128, G, 128] bf16
            selfs = []
            for i in range(2):
                st = spool.tile([128, G, 128], BF16, tag=f"selfs{i}")
                selfs.append(st)
            vtail = v - (G - 1) * 128  # width of the last group
            if vtail < 128:
                nc.vector.memset(selfs[0][:, G - 1, :], 0.0)
                nc.vector.memset(selfs[1][:, G - 1, :], 0.0)

            def wpair_matmul(ps, lhsT, rhs_pair=None, lhs_pair=None,
                             rhs=None):
                """ps = lhsT.T @ (rhs_hi+rhs_lo)  or  (lhs_hi+lhs_lo).T @ rhs."""
                if rhs_pair is not None:
                    hi, lo = rhs_pair
                    nc.tensor.matmul(ps, lhsT, hi, start=True, stop=False)
                    nc.tensor.matmul(ps, lhsT, lo, start=False, stop=True)
                else:
                    hi, lo = lhs_pair
                    nc.tensor.matmul(ps, hi, rhs, start=True, stop=False)
                    nc.tensor.matmul(ps, lo, rhs, start=False, stop=True)

            def hw_scale_store(src_fm, j0, wb, Wp, lid):
                """self/table rows: hb[j0:j0+wb] = dinv * (src_fm.T @ W)."""
                g = j0 // 128
                pA = pmm.tile([128, 128], F32, tag="pA")
                wpair_matmul(pA[:wb, :], src_fm, rhs_pair=Wp)
                hb = selfs[lid][:, g, :]
                nc.scalar.activation(hb[:wb, :], pA[:wb, :], AF.Copy,
                                     scale=dBs[:wb, g:g + 1])

            def store_rows(loc_ab, selft, j0, j1):
                """DMA rows [j0:j1) of the node-major table from selft."""
                loc_a, loc_b = loc_ab
                while j0 < j1:
                    e = min(j1, vh if j0 < vh else j1)
                    loc, off = (loc_a, 0) if j0 < vh else (loc_b, vh)
                    ng = (e - j0) // 128
                    if ng >= 1:
                        dst = loc[j0 - off:j0 - off + ng * 128, :]
                        dst = dst.rearrange("(k p) f -> p k f", p=128)
                        g0 = j0 // 128
                        nc.sync.dma_start(dst, selft[:, g0:g0 + ng, :])
                        j0 += ng * 128
                    else:  # partial tail (< 128 rows)
                        wb = e - j0
                        g0 = j0 // 128
                        nc.sync.dma_start(loc[j0 - off:j0 - off + wb, :],
                                          selft[:wb, g0, :])
                        j0 = e

            # ---- encoder (local nodes, feature-major) + hws0 ----
            pend0 = 0
            ENC_W = 256
            tiles_enc = []
            for hseg, (h0, h1) in enumerate(((0, vh), (vh, v))):
                for a0 in range(h0, h1, ENC_W):
                    tiles_enc.append((hseg, a0, min(ENC_W, h1 - a0)))
            for hseg, a0, w in tiles_enc:
                xoff = a0 - (0 if hseg == 0 else vh)
                xt = xh[hseg][:, xoff:xoff + w]
                p1 = penc1.tile([128, ENC_W], F32, tag="p1")
                nc.tensor.matmul(p1[:, :w], w1hi, xt, start=True, stop=False)
                nc.tensor.matmul(p1[:, :w], w1lo, xt, start=False, stop=True)
                e1 = epool.tile([128, ENC_W], BF16, tag="e1")
                nc.scalar.activation(e1[:, :w], p1[:, :w], AF.Relu,
                                     bias=b1s[:, 0:1])
                p2 = penc2.tile([128, ENC_W], F32, tag="p2")
                nc.tensor.matmul(p2[:, :w], w2hi, e1[:, :w],
                                 start=True, stop=False)
                nc.tensor.matmul(p2[:, :w], w2lo, e1[:, :w],
                                 start=False, stop=True)
                e2 = epool.tile([128, ENC_W], BF16, tag="e2")
                nc.vector.tensor_scalar_add(e2[:, :w], p2[:, :w],
                                            b2s[:, 0:1])
                for j0 in range(0, w, 128):
                    wb = min(128, w - j0)
                    hw_scale_store(e2[:, j0:j0 + wb], a0 + j0, wb,
                                   (g0hi, g0lo), 0)
                # flush finished full groups in >=512-row batches (or at
                # the half/end boundary)
                done = a0 + w
                if done - pend0 >= 512 or done in (vh, v):
                    store_rows((hws0_locA, hws0_locB), selfs[0], pend0, done)
                    pend0 = done

            def allgather(src_ab, dst):
                tb = 0
                for src in src_ab:
                    if src is None:
                        continue
                    w = src[:, :].shape[0]
                    if fake_ag:
                        nc.sync.dma_start(dst[tb:tb + 1, :], src[0:1, :])
                    else:
                        nc.gpsimd.collective_compute(
                            "AllGather", mybir.AluOpType.bypass,
                            replica_groups=rg,
                            ins=[src[:, :]],
                            outs=[dst[tb:tb + ncores * w, :]])
                    tb += ncores * w

            allgather((hws0_locA, hws0_locB), hws0_full)

            # ---- one GCN conv layer: gather + PE accumulate + epilogue ----
            order = sorted(range(G), key=lambda g: -caps[g])
            M_SPLIT = 10  # first groups get an AG-A-only head chunk
            tblA = 8 * vh  # rows [0:tblA) of the table come from AG part A

            def conv(hws, gb_row, lid, Wp_next=None, hws_next=None):
                final = Wp_next is None
                for gi, g in enumerate(order):
                    K = caps[g]
                    boff = base[g]
                    # head chunk: slots [0:kA) reference only table part A,
                    # so this gather depends on AG-A alone; its own small
                    # tile lets many head chunks run before the big tiles
                    # (gated on AG-B) free up.
                    kA = 0
                    if ca is not None and gi < M_SPLIT:
                        kA = min(int(ca[g]), K - 1, KCH, 16)
                        if kA >= 4:
                            gtA = gpoolA.tile([128, 16, 128], BF16,
                                              tag="gtA")
                            nidx = 128 * kA
                            nc.gpsimd.dma_gather(
                                gtA[:, 0:kA, :], hws[:tblA, :],
                                idxs_sb[:, 8 * boff:8 * (boff + kA)],
                                nidx, nidx, H, single_packet=False)
                        else:
                            kA = 0
                    KB = K - kA
                    gt = gpool.tile([128, maxcap, 128], BF16, tag="gt")
                    # chunk the drain group so its accumulation overlaps
                    # the final transfers (shrinks the kernel tail)
                    kch = 8 if gi == G - 1 else KCH
                    off = 0
                    while off < KB:
                        kc = min(KB - off, kch)
                        nidx = 128 * kc
                        bo = boff + kA + off
                        nc.gpsimd.dma_gather(
                            gt[:, off:off + kc, :], hws[:, :],
                            idxs_sb[:, 8 * bo:8 * (bo + kc)],
                            nidx, nidx, H, single_packet=False)
                        off += kc
                    # PSUM accumulation: K gathered slots + self + bias.
                    ps = pagg.tile([128, 128], F32, tag="ps")
                    slots = ([gtA[:, k, :] for k in range(kA)] if kA
                             else []) + [gt[:, k, :] for k in range(KB)]
                    nc.tensor.matmul(ps[:, :], ids, slots[0],
                                     start=True, stop=False)
                    for sl in slots[1:]:
                        nc.tensor.matmul(ps[:, :], ids, sl,
                                         start=False, stop=False)
                    nc.tensor.matmul(ps[:, :], ids, selfs[lid][:, g, :],
                                     start=False, stop=False)
                    # += sqrt(deg)[dst] (x) bias[f]  (rank-1, K=1)
                    nc.tensor.matmul(ps[:, :],
                                     sds[0:1, g * 128:(g + 1) * 128],
                                     gb_row,
                                     start=False, stop=True)
                    wg = min(128, v - g * 128)
                    if final:
                        ob = bpool.tile([128, 128], F32, tag="obf")
                        nc.scalar.activation(ob[:wg, :], ps[:wg, :], AF.Copy,
                                             scale=dBs[:wg, g:g + 1])
                        nc.sync.dma_start(hout[g * 128:g * 128 + wg, :],
                                          ob[:wg, :])
                    else:
                        # relu'd node-major hidden, bf16
                        hrel = bpool.tile([128, 128], BF16, tag="hrel")
                        if wg < 128:
                            nc.vector.memset(hrel[:, :], 0.0)
                        nc.scalar.activation(hrel[:wg, :], ps[:wg, :],
                                             AF.Relu,
                                             scale=dBs[:wg, g:g + 1])
                        # transpose to feature-major (bf16 PSUM out)
                        pT = ptp.tile([128, 128], BF16, tag="pT")
                        nc.tensor.transpose(pT[:, :], hrel[:, :], ids)
                        hfm = bpool.tile([128, 128], BF16, tag="hfm")
                        nc.scalar.activation(hfm[:, :], pT[:, :], AF.Copy)
                        hw_scale_store(hfm[:, :wg], g * 128, wg, Wp_next, 1)
                        store_rows(hws_next, selfs[1], g * 128,
                                   g * 128 + wg)

            conv(hws0_full, gb0s, lid=0, Wp_next=(g1hi, g1lo),
                 hws_next=(hws1_locA, hws1_locB))
            allgather((hws1_locA, hws1_locB), hws1_full)
            conv(hws1_full, gb1s, lid=1)

    nc.compile()
    return nc


# ---------------------------------------------------------------------------
# Host entry point.
# ---------------------------------------------------------------------------

_CACHE = {}


def _get_program(n, v, ncores, caps, ca):
    key = (n, v, ncores, caps, ca)
    prog = _CACHE.get(key)
    if prog is None:
        prog = build_program(n, v, ncores, caps, ca=ca)
        _CACHE[key] = prog
    return prog


def make_in_maps(x, plan, enc_w, gcn_w):
    import ml_dtypes
    BF = ml_dtypes.bfloat16
    n, v, ncores = plan.n, plan.v, plan.ncores
    G = plan.ngroups
    pW1, pb1, pW2, pb2, qW1, qb1, qW2, qb2 = [
        np.ascontiguousarray(np.asarray(a, np.float32)) for a in enc_w]
    gW0, gb0, gW1, gb1 = [
        np.ascontiguousarray(np.asarray(a, np.float32)) for a in gcn_w]
    ident = np.eye(128, dtype=np.float32)

    def split(w):
        hi = w.astype(BF)
        lo = (w - hi.astype(np.float32)).astype(BF)
        return hi, lo

    g0hi, g0lo = split(gW0)
    g1hi, g1lo = split(gW1)
    in_maps = []
    for c in range(ncores):
        rows = plan.pi[c * v:(c + 1) * v]
        is_plant = rows[0] < (n // 2)
        xTc = np.ascontiguousarray(x[rows].T.astype(BF))
        if is_plant:
            w1, b1, w2, b2 = pW1, pb1, pW2, pb2
        else:
            w1, b1, w2, b2 = qW1, qb1, qW2, qb2
        w1hi, w1lo = split(w1)
        w2hi, w2lo = split(w2)
        cb = np.concatenate(
            [w1hi, w1lo, w2hi, w2lo, g0hi, g0lo, g1hi, g1lo,
             ident.astype(BF)], axis=1)
        rowp = np.concatenate(
            [gb0.reshape(1, H), gb1.reshape(1, H),
             plan.sdeg[c]], axis=1).astype(BF)
        cf = np.concatenate(
            [plan.dinv_b[c], b1.reshape(H, 1), b2.reshape(H, 1)],
            axis=1).astype(np.float32)
        in_maps.append({
            "cb": np.ascontiguousarray(cb),
            "rowp": np.ascontiguousarray(rowp),
            "cf": np.ascontiguousarray(cf),
            "idxs": plan.idxs[c],
            "xT": xTc,
        })
    return in_maps


def assemble_output(results, plan):
    n, v = plan.n, plan.v
    out = np.empty((n, H), np.float32)
    for c in range(plan.ncores):
        out[plan.pi[c * v:(c + 1) * v]] = results[c]["hout"]
    return out


def kernel(**inputs):
    x = np.asarray(inputs["x"], np.float32)
    ei = np.asarray(inputs["edge_index"], np.int64)
    assert x.shape == (N_NODES, H) and ei.shape == (2, N_EDGES)
    assert int(inputs["num_plants"]) == N_PLANTS

    plan = build_plan(ei[0], ei[1], N_NODES, V_PER_CORE, N_CORES)
    nc = _get_program(N_NODES, V_PER_CORE, N_CORES, plan.caps, plan.ca)

    enc_w = (inputs["pW1"], inputs["pb1"], inputs["pW2"], inputs["pb2"],
             inputs["qW1"], inputs["qb1"], inputs["qW2"], inputs["qb2"])
    gcn_w = (inputs["gW0"], inputs["gb0"], inputs["gW1"], inputs["gb1"])
    in_maps = make_in_maps(x, plan, enc_w, gcn_w)

    from concourse.bass_utils import run_bass_kernel_spmd
    res = run_bass_kernel_spmd(nc, in_maps, list(range(N_CORES)))
    return assemble_output(res.results, plan)
